# revision 6
# baseline (speedup 1.0000x reference)
"""Distributed GCN ActorNetwork kernel for 8 Trainium2 NeuronCores.

Strategy:
  - Nodes sharded contiguously across 8 cores (dst-owner partitioning).
  - Both GCN layers aggregate 32-dim vectors: layer 2 uses
    A_hat @ (z W2) == (A_hat @ z) W2, so the gather tables are [N, 32].
  - Normalization factorized: out = dinv * (sum_{e->d} hs[src] + hs[d]) + b
    with hs = dinv * (x @ W1)  (resp. zs1 = dinv * z1 for layer 2).
  - Per layer: AllGather of the scaled node table (padded to 64 f32 cols for
    dma_gather's 256-byte row-stride requirement), then per-edge rows are
    fetched with dma_gather (4 parallel SWDGE queues), and segment-sums are
    done with one-hot matrices (DVE is_equal vs iota) x TensorE matmuls
    accumulating in PSUM per 64-node dst subtile.
  - MLP head computed feature-major per subtile; global softmax via two
    scalar AllReduces (max, sum).
"""
import sys
sys.path.insert(0, '/opt/trn_rl_repo')

import numpy as np

NC_ = 8           # cores
P = 128           # SBUF partitions
W = 64            # dst-subtile width (nodes)
CHUNK = 32768     # gather-table chunk (int16 index limit)
GB = 4            # subtiles per gather block
ELEM = 64         # gather-table row f32 elements (256 B)


def _host_prep(x, src, dst, N, dims):
    IN, H1, H2, A = dims
    assert N % NC_ == 0
    Sc = N // NC_
    nsub = -(-Sc // W)
    ncc = -(-N // CHUNK)

    deg = np.bincount(dst, minlength=N).astype(np.float64) + 1.0
    dinv = (1.0 / np.sqrt(deg)).astype(np.float32)

    owner = dst // Sc
    drel = dst % Sc
    s_all = drel // W
    q_all = drel % W
    cc_all = src // CHUNK

    # shared schedule: cols per (subtile, chunk) = max over cores
    key_full = ((owner * nsub) + s_all) * ncc + cc_all
    counts = np.bincount(key_full, minlength=NC_ * nsub * ncc).reshape(NC_, nsub, ncc)
    cols_sc = -(-counts.max(axis=0) // P)          # [nsub, ncc]
    cols_sc = np.maximum(cols_sc, 1)               # keep >=1 col per segment

    # call order: blocks of GB subtiles, then cc; within call subtiles asc.
    nblk = -(-nsub // GB)
    colbase = np.zeros((nsub, ncc), np.int64)
    calls = []     # (cc, [(s, colbase, cols)...], callbase, cols_call)
    run = 0
    for b in range(nblk):
        subs = range(b * GB, min((b + 1) * GB, nsub))
        for cc in range(ncc):
            cb = run
            seglist = []
            for s in subs:
                colbase[s, cc] = run
                seglist.append((s, run, int(cols_sc[s, cc])))
                run += int(cols_sc[s, cc])
            calls.append((cc, seglist, cb, run - cb))
    totcols = run
    totslots = totcols * P

    segslotbase = (colbase * P).reshape(-1)        # by key s*ncc+cc

    per_core = []
    for c in range(NC_):
        m = owner == c
        src_c = src[m]
        key = (s_all[m] * ncc + cc_all[m]).astype(np.int64)
        order = np.argsort(key, kind='stable')
        key_s = key[order]
        grp_counts = np.bincount(key_s, minlength=nsub * ncc)
        grp_starts = np.concatenate([[0], np.cumsum(grp_counts)[:-1]])
        rank = np.arange(len(key_s), dtype=np.int64) - grp_starts[key_s]
        slot = segslotbase[key_s] + rank
        idx_flat = np.zeros(totslots, np.int16)
        idx_flat[slot] = (src_c[order] % CHUNK).astype(np.int16)
        drel_flat = np.full(totslots, -1.0, np.float32)
        drel_flat[slot] = q_all[m][order].astype(np.float32)

        # wrap-16 index layout, replicated across the 8 groups of 16 partitions
        iw = idx_flat.reshape(totcols * 8, 16).T    # [16, totcols*8]
        idx_w = np.tile(iw, (8, 1))                 # [128, totcols*8]
        dstrel = np.ascontiguousarray(drel_flat.reshape(totcols, P).T)  # [128, totcols]

        base = c * Sc
        npad = nsub * W
        xT = np.zeros((IN, npad), np.float32)
        xT[:, :Sc] = x[base:base + Sc].T
        dv = np.ones(npad, np.float32)
        dv[:Sc] = dinv[base:base + Sc]
        dinv_sub = np.ascontiguousarray(dv.reshape(nsub, W).T)  # [W, nsub]

        per_core.append(dict(xT=xT, idx=idx_w, dstrel=dstrel, dinv_sub=dinv_sub))

    sched = dict(Sc=Sc, nsub=nsub, ncc=ncc, nblk=nblk, calls=calls,
                 totcols=totcols, cols_sc=cols_sc)
    return per_core, sched


def _build(sched, dims):
    from concourse import bass, bacc, mybir, tile

    IN, H1, H2, A = dims
    Sc, nsub, ncc = sched["Sc"], sched["nsub"], sched["ncc"]
    calls = sched["calls"]
    totcols = sched["totcols"]
    npad = nsub * W
    fdt = mybir.dt.float32
    NTAB = ncc * CHUNK

    nc = bacc.Bacc("TRN2", target_bir_lowering=False, debug=False,
                   num_swdge_queues=min(4, ncc))

    xT_e = nc.declare_dram_parameter("xT", [IN, npad], fdt, isOutput=False)
    dinv_e = nc.declare_dram_parameter("dinv_sub", [W, nsub], fdt, isOutput=False)
    idx_e = nc.declare_dram_parameter("idx", [P, totcols * 8], mybir.dt.int16, isOutput=False)
    drel_e = nc.declare_dram_parameter("dstrel", [P, totcols], fdt, isOutput=False)
    w1_e = nc.declare_dram_parameter("w1", [IN, H1], fdt, isOutput=False)
    w2_e = nc.declare_dram_parameter("w2", [H1, H2], fdt, isOutput=False)
    wo1_e = nc.declare_dram_parameter("wo1", [H2, H2], fdt, isOutput=False)
    wo2_e = nc.declare_dram_parameter("wo2", [H2, A], fdt, isOutput=False)
    b1_e = nc.declare_dram_parameter("b1rep", [W, H1], fdt, isOutput=False)
    b2_e = nc.declare_dram_parameter("b2col", [H2, 1], fdt, isOutput=False)
    bo1_e = nc.declare_dram_parameter("bo1col", [H2, 1], fdt, isOutput=False)
    bo2_e = nc.declare_dram_parameter("bo2col", [A, 1], fdt, isOutput=False)
    iota_e = nc.declare_dram_parameter("iota", [P, W], fdt, isOutput=False)
    id64_e = nc.declare_dram_parameter("id64", [W, W], fdt, isOutput=False)
    ones1_e = nc.declare_dram_parameter("ones1", [1, W], fdt, isOutput=False)
    out_e = nc.declare_dram_parameter("out", [A, npad], fdt, isOutput=True)

    with tile.TileContext(nc) as tc:
        with tc.tile_pool(name="dram", bufs=1, space="DRAM") as dramp, \
             tc.tile_pool(name="const", bufs=1) as cp, \
             tc.tile_pool(name="stage", bufs=3) as stp, \
             tc.tile_pool(name="gath", bufs=4) as gp, \
             tc.tile_pool(name="work", bufs=4) as wp, \
             tc.tile_pool(name="ps", bufs=1, space="PSUM") as pp, \
             tc.tile_pool(name="psagg", bufs=4, space="PSUM") as pagg:

            ag1 = dramp.tile([npad, ELEM], fdt)
            ag2 = dramp.tile([npad, ELEM], fdt)
            tab1 = dramp.tile([NTAB, ELEM], fdt)
            tab2 = dramp.tile([NTAB, ELEM], fdt)
            ar_in = dramp.tile([1, 1], fdt)
            ar_out = dramp.tile([1, 1], fdt)
            ar_in2 = dramp.tile([1, 1], fdt)
            ar_out2 = dramp.tile([1, 1], fdt)

            # ---- constants ----
            w1a = cp.tile([P, H1], fdt)
            w1b = cp.tile([P, H1], fdt)
            nc.sync.dma_start(out=w1a[:], in_=w1_e[0:P, :])
            nc.sync.dma_start(out=w1b[:], in_=w1_e[P:2 * P, :])
            w2s = cp.tile([H1, H2], fdt)
            nc.sync.dma_start(out=w2s[:], in_=w2_e[:, :])
            wo1s = cp.tile([H2, H2], fdt)
            nc.sync.dma_start(out=wo1s[:], in_=wo1_e[:, :])
            wo2s = cp.tile([H2, A], fdt)
            nc.sync.dma_start(out=wo2s[:], in_=wo2_e[:, :])
            b1s = cp.tile([W, H1], fdt)
            nc.sync.dma_start(out=b1s[:], in_=b1_e[:, :])
            b2s = cp.tile([H2, 1], fdt)
            nc.sync.dma_start(out=b2s[:], in_=b2_e[:, :])
            bo1s = cp.tile([H2, 1], fdt)
            nc.sync.dma_start(out=bo1s[:], in_=bo1_e[:, :])
            bo2s = cp.tile([A, 1], fdt)
            nc.sync.dma_start(out=bo2s[:], in_=bo2_e[:, :])
            iotas = cp.tile([P, W], fdt)
            nc.sync.dma_start(out=iotas[:], in_=iota_e[:, :])
            id64s = cp.tile([W, W], fdt)
            nc.sync.dma_start(out=id64s[:], in_=id64_e[:, :])
            ones1s = cp.tile([1, W], fdt)
            nc.sync.dma_start(out=ones1s[:], in_=ones1_e[:, :])
            dinvs = cp.tile([W, nsub], fdt)
            nc.sync.dma_start(out=dinvs[:], in_=dinv_e[:, :])
            drels = cp.tile([P, totcols], fdt)
            nc.sync.dma_start(out=drels[:], in_=drel_e[:, :])

            hs1o = cp.tile([W, nsub * H1], fdt)   # resident self-terms
            zs1o = cp.tile([W, nsub * H1], fdt)
            logr = cp.tile([A, nsub * W], fdt)    # resident logits^T
            rmax = cp.tile([A, 1], fdt)
            esum = cp.tile([A, 1], fdt)
            nc.vector.memset(rmax[:], -3.0e38)
            nc.vector.memset(esum[:], 0.0)

            # ---- stage A: hs1 = dinv * (x @ W1), fill ag1 ----
            nblkA = -(-nsub // GB)
            for b in range(nblkA):
                subs = list(range(b * GB, min((b + 1) * GB, nsub)))
                w_cols = len(subs) * W
                c0 = subs[0] * W
                xa = stp.tile([P, GB * W], fdt, tag="xa")
                xb = stp.tile([P, GB * W], fdt, tag="xb")
                nc.sync.dma_start(out=xa[:, :w_cols], in_=xT_e[0:P, c0:c0 + w_cols])
                nc.sync.dma_start(out=xb[:, :w_cols], in_=xT_e[P:2 * P, c0:c0 + w_cols])
                for si, s in enumerate(subs):
                    h1p = pp.tile([W, H1], fdt, space="PSUM", tag="mm1")
                    nc.tensor.matmul(out=h1p[:], lhsT=xa[:, si * W:(si + 1) * W],
                                     rhs=w1a[:], start=True, stop=False)
                    nc.tensor.matmul(out=h1p[:], lhsT=xb[:, si * W:(si + 1) * W],
                                     rhs=w1b[:], start=False, stop=True)
                    nc.vector.tensor_scalar_mul(
                        out=hs1o[:, s * H1:(s + 1) * H1], in0=h1p[:],
                        scalar1=dinvs[:, s:s + 1])
                    nc.scalar.dma_start(out=ag1[s * W:(s + 1) * W, 0:H1],
                                        in_=hs1o[:, s * H1:(s + 1) * H1])

            nc.gpsimd.collective_compute(
                "AllGather", mybir.AluOpType.bypass,
                replica_groups=[list(range(NC_))],
                ins=[ag1[0:Sc, :].opt()],
                outs=[tab1[0:Sc * NC_, :].opt()],
            )

            # ---- aggregation layers ----
            def agg_layer(table, selfo, epilogue):
                ic = 0
                psums = {}
                seg_idx = {}
                for s in range(nsub):
                    seg_idx[s] = 0
                for (cc, seglist, cb, cols_call) in calls:
                    idx_t = gp.tile([P, cols_call * 8], mybir.dt.int16, tag="idx")
                    nc.scalar.dma_start(out=idx_t[:],
                                        in_=idx_e[:, cb * 8:(cb + cols_call) * 8])
                    msgs = gp.tile([P, cols_call * ELEM], fdt, tag="msgs")
                    m3 = msgs[:].rearrange("p (j e) -> p j e", e=ELEM)
                    nc.gpsimd.dma_gather(m3, table[cc * CHUNK:(cc + 1) * CHUNK, :],
                                         idx_t[:, :], cols_call * P, cols_call * P,
                                         ELEM, single_packet=False, queue_num=cc)
                    sb = gp.tile([P, cols_call * W], fdt, tag="sbuild")
                    s3o = sb[:].rearrange("p (j w) -> p j w", w=W)
                    in0 = drels[:, cb:cb + cols_call].unsqueeze(2) \
                        .to_broadcast([P, cols_call, W])
                    in1 = iotas[:].unsqueeze(1) \
                        .to_broadcast([P, cols_call, W])
                    nc.vector.tensor_tensor(out=s3o, in0=in0, in1=in1,
                                            op=mybir.AluOpType.is_equal)
                    for (s, scb, scols) in seglist:
                        if s not in psums:
                            psums[s] = pagg.tile([W, H1], fdt, space="PSUM", tag="agg", name=f"agg{s}")
                        for j in range(scols):
                            jl = (scb - cb) + j
                            first = (seg_idx[s] == 0 and j == 0)
                            last = (cc == ncc - 1 and j == scols - 1)
                            nc.tensor.matmul(
                                out=psums[s][:],
                                lhsT=sb[:, jl * W:(jl + 1) * W],
                                rhs=msgs[:, jl * ELEM:jl * ELEM + H1],
                                start=first, stop=last)
                        seg_idx[s] += 1
                        if cc == ncc - 1:
                            epilogue(s, psums.pop(s))

            # ---- L1 epilogue: z1 relu, zs1 = dinv*z1 -> ag2 + resident ----
            def epi1(s, psum):
                u = wp.tile([W, H1], fdt, tag="u1")
                nc.vector.tensor_tensor(out=u[:], in0=psum[:],
                                        in1=hs1o[:, s * H1:(s + 1) * H1],
                                        op=mybir.AluOpType.add)
                nc.vector.tensor_scalar_mul(out=u[:], in0=u[:],
                                            scalar1=dinvs[:, s:s + 1])
                nc.vector.tensor_tensor(out=u[:], in0=u[:], in1=b1s[:],
                                        op=mybir.AluOpType.add)
                z = wp.tile([W, H1], fdt, tag="z1")
                nc.scalar.activation(out=z[:], in_=u[:],
                                     func=mybir.ActivationFunctionType.Relu)
                nc.scalar.activation(out=zs1o[:, s * H1:(s + 1) * H1], in_=z[:],
                                     func=mybir.ActivationFunctionType.Copy,
                                     scale=dinvs[:, s:s + 1])
                nc.scalar.dma_start(out=ag2[s * W:(s + 1) * W, 0:H1],
                                    in_=zs1o[:, s * H1:(s + 1) * H1])

            agg_layer(tab1, hs1o, epi1)

            nc.gpsimd.collective_compute(
                "AllGather", mybir.AluOpType.bypass,
                replica_groups=[list(range(NC_))],
                ins=[ag2[0:Sc, :].opt()],
                outs=[tab2[0:Sc * NC_, :].opt()],
            )

            # ---- L2 epilogue: v=dinv*(agg+zs1); head; logits resident ----
            def epi2(s, psum):
                u = wp.tile([W, H1], fdt, tag="u2")
                nc.vector.tensor_tensor(out=u[:], in0=psum[:],
                                        in1=zs1o[:, s * H1:(s + 1) * H1],
                                        op=mybir.AluOpType.add)
                v = wp.tile([W, H1], fdt, tag="v2")
                nc.vector.tensor_scalar_mul(out=v[:], in0=u[:],
                                            scalar1=dinvs[:, s:s + 1])
                tp = pp.tile([H1, W], fdt, space="PSUM", tag="tp")
                nc.tensor.transpose(out=tp[:], in_=v[:], identity=id64s[:])
                vT = wp.tile([H1, W], fdt, tag="vT")
                nc.vector.tensor_copy(out=vT[:], in_=tp[:])
                mm = pp.tile([H2, W], fdt, space="PSUM", tag="mm2")
                nc.tensor.matmul(out=mm[:], lhsT=w2s[:], rhs=vT[:],
                                 start=True, stop=True)
                z2 = wp.tile([H2, W], fdt, tag="z2")
                nc.scalar.activation(out=z2[:], in_=mm[:],
                                     func=mybir.ActivationFunctionType.Relu,
                                     bias=b2s[:, 0:1])
                mm2 = pp.tile([H2, W], fdt, space="PSUM", tag="mm1")
                nc.tensor.matmul(out=mm2[:], lhsT=wo1s[:], rhs=z2[:],
                                 start=True, stop=True)
                a1 = wp.tile([H2, W], fdt, tag="a1")
                nc.scalar.activation(out=a1[:], in_=mm2[:],
                                     func=mybir.ActivationFunctionType.Relu,
                                     bias=bo1s[:, 0:1])
                mm3 = pp.tile([A, W], fdt, space="PSUM", tag="mm2")
                nc.tensor.matmul(out=mm3[:], lhsT=wo2s[:], rhs=a1[:],
                                 start=True, stop=True)
                nc.vector.tensor_tensor(out=logr[:, s * W:(s + 1) * W],
                                        in0=mm3[:],
                                        in1=bo2s[:, 0:1].to_broadcast([A, W]),
                                        op=mybir.AluOpType.add)
                mx = wp.tile([A, 1], fdt, tag="mx")
                nc.vector.tensor_reduce(out=mx[:],
                                        in_=logr[:, s * W:(s + 1) * W],
                                        axis=mybir.AxisListType.X,
                                        op=mybir.AluOpType.max)
                nc.vector.tensor_tensor(out=rmax[:], in0=rmax[:], in1=mx[:],
                                        op=mybir.AluOpType.max)

            agg_layer(tab2, zs1o, epi2)

            # ---- softmax ----
            def part_reduce(col, op):
                # [A,1] -> scalar [1,1] via transpose + free reduce
                tp = pp.tile([H1, W], fdt, space="PSUM", tag="tp")
                nc.tensor.transpose(out=tp[0:1, 0:A], in_=col[:], identity=id64s[:])
                row = wp.tile([1, A], fdt, tag="row")
                nc.vector.tensor_copy(out=row[:], in_=tp[0:1, 0:A])
                sc = wp.tile([1, 1], fdt, tag="sc")
                nc.vector.tensor_reduce(out=sc[:], in_=row[:],
                                        axis=mybir.AxisListType.X, op=op)
                return sc

            def bcast_col(scalar_sb, tag):
                # [1,1] -> [A,1] replicated via ones outer-product matmul
                bp = pp.tile([W, 1], fdt, space="PSUM", tag="tp")
                nc.tensor.matmul(out=bp[0:A, :], lhsT=ones1s[:, 0:A],
                                 rhs=scalar_sb[:], start=True, stop=True)
                col = wp.tile([A, 1], fdt, tag=tag)
                nc.vector.tensor_copy(out=col[:], in_=bp[0:A, :])
                return col

            mloc = part_reduce(rmax, mybir.AluOpType.max)
            nc.sync.dma_start(out=ar_in[:, :], in_=mloc[:])
            nc.gpsimd.collective_compute(
                "AllReduce", mybir.AluOpType.max,
                replica_groups=[list(range(NC_))],
                ins=[ar_in[:, :].opt()], outs=[ar_out[:, :].opt()])
            gmax = wp.tile([1, 1], fdt, tag="gmax")
            nc.sync.dma_start(out=gmax[:], in_=ar_out[:, :])
            ngmax = wp.tile([1, 1], fdt, tag="ngmax")
            nc.vector.tensor_scalar_mul(out=ngmax[:], in0=gmax[:], scalar1=-1.0)
            ngcol = bcast_col(ngmax, "ngcol")

            for s in range(nsub):
                sl = logr[:, s * W:(s + 1) * W]
                nc.scalar.activation(out=sl, in_=sl,
                                     func=mybir.ActivationFunctionType.Exp,
                                     bias=ngcol[:, 0:1])
                es = wp.tile([A, 1], fdt, tag="es")
                nc.vector.tensor_reduce(out=es[:], in_=sl,
                                        axis=mybir.AxisListType.X,
                                        op=mybir.AluOpType.add)
                nc.vector.tensor_tensor(out=esum[:], in0=esum[:], in1=es[:],
                                        op=mybir.AluOpType.add)

            sloc = part_reduce(esum, mybir.AluOpType.add)
            nc.sync.dma_start(out=ar_in2[:, :], in_=sloc[:])
            nc.gpsimd.collective_compute(
                "AllReduce", mybir.AluOpType.add,
                replica_groups=[list(range(NC_))],
                ins=[ar_in2[:, :].opt()], outs=[ar_out2[:, :].opt()])
            gsum = wp.tile([1, 1], fdt, tag="gsum")
            nc.sync.dma_start(out=gsum[:], in_=ar_out2[:, :])
            rinv = wp.tile([1, 1], fdt, tag="rinv")
            nc.vector.reciprocal(out=rinv[:], in_=gsum[:])
            rcol = bcast_col(rinv, "rcol")

            for s in range(nsub):
                ot = wp.tile([A, W], fdt, tag="ot")
                nc.vector.tensor_scalar_mul(out=ot[:],
                                            in0=logr[:, s * W:(s + 1) * W],
                                            scalar1=rcol[:, 0:1])
                nc.scalar.dma_start(out=out_e[:, s * W:(s + 1) * W], in_=ot[:])

    return nc


def _run(nc, in_maps):
    from concourse.bass_utils import run_bass_kernel_spmd
    res = run_bass_kernel_spmd(nc, in_maps, core_ids=list(range(NC_)))
    return res.results


_CACHE = {}


def build_all(inputs):
    x = np.ascontiguousarray(np.asarray(inputs["x"], dtype=np.float32))
    ei = np.asarray(inputs["ei"])
    N = int(np.asarray(inputs["num_nodes"]))
    W1 = np.asarray(inputs["W1"], np.float32); b1 = np.asarray(inputs["b1"], np.float32)
    W2 = np.asarray(inputs["W2"], np.float32); b2 = np.asarray(inputs["b2"], np.float32)
    Wo1 = np.asarray(inputs["Wo1"], np.float32); bo1 = np.asarray(inputs["bo1"], np.float32)
    Wo2 = np.asarray(inputs["Wo2"], np.float32); bo2 = np.asarray(inputs["bo2"], np.float32)
    IN, H1 = W1.shape
    H2 = W2.shape[1]
    A = Wo2.shape[1]
    dims = (IN, H1, H2, A)
    assert IN == 2 * P and H1 <= W and H2 <= W and A <= W

    src = np.asarray(ei[0], np.int64)
    dst = np.asarray(ei[1], np.int64)
    per_core, sched = _host_prep(x, src, dst, N, dims)

    iota = np.tile(np.arange(W, dtype=np.float32), (P, 1))
    id64 = np.eye(W, dtype=np.float32)
    ones1 = np.ones((1, W), np.float32)
    b1rep = np.tile(b1[None, :], (W, 1)).astype(np.float32)
    b2col = b2[:, None].astype(np.float32)
    bo1col = bo1[:, None].astype(np.float32)
    bo2col = bo2[:, None].astype(np.float32)

    in_maps = []
    for c in range(NC_):
        pc = per_core[c]
        in_maps.append({
            "xT": pc["xT"], "dinv_sub": pc["dinv_sub"], "idx": pc["idx"],
            "dstrel": pc["dstrel"],
            "w1": W1, "w2": W2, "wo1": Wo1, "wo2": Wo2,
            "b1rep": b1rep, "b2col": b2col, "bo1col": bo1col, "bo2col": bo2col,
            "iota": iota, "id64": id64, "ones1": ones1,
        })

    nc = _build(sched, dims)
    nc.finalize()
    return nc, in_maps, sched, dims, N


def kernel(**inputs):
    nc, in_maps, sched, dims, N = build_all(inputs)
    results = _run(nc, in_maps)
    Sc = sched["Sc"]
    A = dims[3]
    parts = [results[c]["out"][:, :Sc].T for c in range(NC_)]
    probs = np.concatenate(parts, axis=0)            # [N, A]
    return probs.reshape(1, N * A).astype(np.float32)
